# revision 2
# baseline (speedup 1.0000x reference)
"""MoE expert-parallel FFN kernel for TRN2 (8 NeuronCores).

Reference computation (per expert e):
    h = gelu(x_e @ W1[e] + b1[e]);  y_e = h @ W2[e] + b2[e]
with x = inputs[0].reshape(E, CAP, D), E=8, CAP=4096, D=1024, F=4096.

Sharding: expert parallel — core e owns expert e and its CAP-token slice.
No cross-core communication.

Per-core dataflow (all matmuls in float32r = full PE rate, ~1e-4 rel err):
  mm1: hT[f, tok] = W1[d, f].T @ xT[d, tok]   (K=D, lhsT=W1 natural layout)
       fused bias+gelu on psum eviction (ACT, per-partition bias = b1)
  mm2: y[tok, d] = hT[f, tok].T @ W2[f, d]    (K=F, lhsT=hT as produced)
       b2 added by DVE at psum eviction.
The host pre-transposes x to xT and pre-tiles W1 so every DMA is
partition-contiguous.
"""

import sys

if "/opt/trn_rl_repo" not in sys.path:
    sys.path.insert(0, "/opt/trn_rl_repo")

from contextlib import ExitStack

import numpy as np

import concourse.bacc as bacc
import concourse.tile as tile
from concourse import mybir
from concourse.bass_utils import run_bass_kernel_spmd

E, CAP, D, F = 8, 4096, 1024, 4096
P = 128
TC = 1024            # tokens per tile
NT = CAP // TC       # token tiles per core
KD = D // P          # k-tiles for mm1
FM = F // P          # f chunks
TM = TC // P         # token sub-tiles for mm2
NDH = D // 512       # output d halves

F32 = mybir.dt.float32
F32R = mybir.dt.float32r
GELU = mybir.ActivationFunctionType.Gelu_apprx_tanh

_cache = {}


def _build():
    nc = bacc.Bacc("TRN2", target_bir_lowering=False, debug=False)

    xt = nc.dram_tensor("xt", [D, CAP], F32R, kind="ExternalInput")
    w1t = nc.dram_tensor("w1t", [FM, P, KD, P], F32R, kind="ExternalInput")
    w2 = nc.dram_tensor("w2", [F, D], F32R, kind="ExternalInput")
    b1t = nc.dram_tensor("b1t", [P, FM], F32, kind="ExternalInput")
    b2b = nc.dram_tensor("b2b", [P, D], F32, kind="ExternalInput")
    y = nc.dram_tensor("y", [CAP, D], F32, kind="ExternalOutput")

    with tile.TileContext(nc) as tc:
        with ExitStack() as ctx:
            const = ctx.enter_context(tc.tile_pool(name="const", bufs=1))
            xpool = ctx.enter_context(tc.tile_pool(name="x", bufs=1))
            htpool = ctx.enter_context(tc.tile_pool(name="ht", bufs=1))
            w1pool = ctx.enter_context(tc.tile_pool(name="w1", bufs=2))
            w2pool = ctx.enter_context(tc.tile_pool(name="w2", bufs=4))
            ypool = ctx.enter_context(tc.tile_pool(name="yev", bufs=2))
            psum = ctx.enter_context(tc.tile_pool(name="psum", bufs=8, space="PSUM"))

            b1_sb = const.tile([P, FM], F32)
            nc.sync.dma_start(b1_sb[:], b1t.ap())
            b2_sb = const.tile([P, D], F32)
            nc.sync.dma_start(b2_sb[:], b2b.ap())

            xt_r = xt.ap().rearrange("(k p) c -> p k c", p=P)  # [128, KD, CAP]
            w1_r = w1t.ap()  # [FM, P, KD, P]
            w2_r = w2.ap()
            y_r = y.ap()

            for t in range(NT):
                x_sb = xpool.tile([P, KD, TC], F32R, tag="x")
                nc.sync.dma_start(x_sb[:], xt_r[:, :, t * TC:(t + 1) * TC])

                ht_sb = htpool.tile([P, FM, TC], F32R, tag="ht")

                # --- mm1: hT[f_chunk, tok] += W1.T @ xT, fused bias+gelu ---
                for fm in range(FM):
                    w1_sb = w1pool.tile([P, KD, P], F32R, tag="w1")
                    nc.sync.dma_start(w1_sb[:], w1_r[fm])
                    for h in range(TC // 512):
                        ps = psum.tile([P, 512], F32, tag="ps")
                        for k in range(KD):
                            nc.tensor.matmul(
                                ps[:],
                                w1_sb[:, k],
                                x_sb[:, k, h * 512:(h + 1) * 512],
                                start=(k == 0),
                                stop=(k == KD - 1),
                            )
                        nc.scalar.activation(
                            ht_sb[:, fm, h * 512:(h + 1) * 512],
                            ps[:],
                            GELU,
                            bias=b1_sb[:, fm:fm + 1],
                        )

                # --- mm2: y[tok, d] += hT.T @ W2, b2 added on eviction ---
                for dh in range(NDH):
                    ps_y = [
                        psum.tile([P, 512], F32, tag="ps", name=f"psy_{t}_{dh}_{i}")
                        for i in range(TM)
                    ]
                    for fm in range(FM):
                        w2_sb = w2pool.tile([P, 512], F32R, tag="w2")
                        nc.sync.dma_start(
                            w2_sb[:],
                            w2_r[fm * P:(fm + 1) * P, dh * 512:(dh + 1) * 512],
                        )
                        for tm in range(TM):
                            nc.tensor.matmul(
                                ps_y[tm][:],
                                ht_sb[:, fm, tm * P:(tm + 1) * P],
                                w2_sb[:],
                                start=(fm == 0),
                                stop=(fm == FM - 1),
                            )
                    for tm in range(TM):
                        y_sb = ypool.tile([P, 512], F32, tag="y")
                        nc.vector.tensor_add(
                            y_sb[:], ps_y[tm][:], b2_sb[:, dh * 512:(dh + 1) * 512]
                        )
                        nc.sync.dma_start(
                            y_r[t * TC + tm * P:t * TC + (tm + 1) * P,
                                dh * 512:(dh + 1) * 512],
                            y_sb[:],
                        )

    nc.compile()
    return nc


def _prep_core_inputs(inputs, W1, b1, W2, b2, e):
    x_e = inputs[0, e * CAP:(e + 1) * CAP, :]          # [CAP, D]
    xt = np.ascontiguousarray(x_e.T)                   # [D, CAP]
    # W1[e]: [D, F] -> [FM, P(d-part... see kernel), KD, P]
    # kernel reads w1t[fm][p, k, f] == W1[k*P + p, fm*P + f]
    w1t = np.ascontiguousarray(
        W1[e].reshape(KD, P, FM, P).transpose(2, 1, 0, 3)
    )
    b1t = np.ascontiguousarray(b1[e].reshape(FM, P).T)  # [P, FM]
    b2b = np.ascontiguousarray(np.broadcast_to(b2[e], (P, D)))
    return {
        "xt": xt,
        "w1t": w1t,
        "w2": np.ascontiguousarray(W2[e]),
        "b1t": b1t,
        "b2b": b2b,
    }


def get_nc():
    if "nc" not in _cache:
        _cache["nc"] = _build()
    return _cache["nc"]


def make_in_maps(inputs, W1, b1, W2, b2):
    inputs = np.asarray(inputs, dtype=np.float32)
    W1 = np.asarray(W1, dtype=np.float32)
    b1 = np.asarray(b1, dtype=np.float32)
    W2 = np.asarray(W2, dtype=np.float32)
    b2 = np.asarray(b2, dtype=np.float32)
    return [_prep_core_inputs(inputs, W1, b1, W2, b2, e) for e in range(E)]


def kernel(inputs, W1, b1, W2, b2):
    nc = get_nc()
    in_maps = make_in_maps(inputs, W1, b1, W2, b2)
    res = run_bass_kernel_spmd(nc, in_maps, list(range(E))).results
    out = np.empty((1, E * CAP, D), dtype=np.float32)
    for e in range(E):
        out[0, e * CAP:(e + 1) * CAP, :] = res[e]["y"]
    return out


if __name__ == "__main__":
    rng = np.random.default_rng(0)
    ins = {
        "inputs": rng.standard_normal((1, E * CAP, D), dtype=np.float32),
        "W1": rng.standard_normal((E, D, F), dtype=np.float32) / np.sqrt(D),
        "b1": np.zeros((E, F), np.float32),
        "W2": rng.standard_normal((E, F, D), dtype=np.float32) / np.sqrt(F),
        "b2": np.zeros((E, D), np.float32),
    }
    y = kernel(**ins)
    print("out", y.shape, y.dtype, float(np.abs(y).mean()))


# revision 12
# speedup vs baseline: 7.6689x; 7.6689x over previous
"""MoE expert-parallel FFN kernel for TRN2 (8 NeuronCores).

Reference computation (per expert e):
    h = gelu(x_e @ W1[e] + b1[e]);  y_e = h @ W2[e] + b2[e]
with x = inputs[0].reshape(E, CAP, D), E=8, CAP=4096, D=1024, F=4096.

Sharding: expert parallel — core e owns expert e and its CAP-token slice.
No cross-core communication.

Per-core dataflow (all matmuls in float32r = full PE rate, ~1e-4 rel err):
  mm1: hT[f, tok] = W1[d, f].T @ xT[d, tok]   (K=D, lhsT=W1 natural layout)
       fused bias+gelu on psum eviction (ACT, per-partition bias = b1)
  mm2: y[tok, d] = hT[f, tok].T @ W2[f, d]    (K=F, lhsT=hT as produced)
       b2 added by DVE at psum eviction.
The host pre-transposes x to xT and pre-tiles W1 so every DMA is
partition-contiguous.
"""

import sys

if "/opt/trn_rl_repo" not in sys.path:
    sys.path.insert(0, "/opt/trn_rl_repo")

from contextlib import ExitStack

import numpy as np

import concourse.bacc as bacc
import concourse.tile as tile
from concourse import mybir
from concourse.bass_utils import run_bass_kernel_spmd

E, CAP, D, F = 8, 4096, 1024, 4096
P = 128
TC = 1024            # tokens per tile
NT = CAP // TC       # token tiles per core
KD = D // P          # k-tiles for mm1
FM = F // P          # f chunks
TM = TC // P         # token sub-tiles for mm2
NDH = D // 512       # output d halves

F32 = mybir.dt.float32
F32R = mybir.dt.float32r
BF16 = mybir.dt.bfloat16
GELU = mybir.ActivationFunctionType.Gelu_apprx_tanh

MM_DT = F32R  # matmul operand dtype: F32R (accurate) or BF16 (fast)
LDW_OPT = False  # walrus redundant-LDW elision wedges the device (NRT_EXEC_UNIT_UNRECOVERABLE)

_cache = {}


def _install_ldw_opt_patch():
    """Flip walrus's --enable-ldw-opt flag (redundant weight-load elision).

    Consecutive matmuls in this kernel share stationary operands; eliding
    the second LDWEIGHTS removes dead weight-load time on the PE.
    """
    import concourse.bass_utils as bu

    if getattr(bu, "_ldw_opt_patched", False):
        return
    orig = bu.run_command

    def patched(cmd, **kw):
        if LDW_OPT and isinstance(cmd, list):
            cmd = ["--enable-ldw-opt=true" if c == "--enable-ldw-opt=false" else c
                   for c in cmd]
        return orig(cmd, **kw)

    bu.run_command = patched
    bu._ldw_opt_patched = True


_install_ldw_opt_patch()


def _build(mm_dt=None, repeat=1, mm1_pair=True):
    if mm_dt is None:
        mm_dt = MM_DT
    nc = bacc.Bacc("TRN2", target_bir_lowering=False, debug=False)

    xt = nc.dram_tensor("xt", [D, CAP], mm_dt, kind="ExternalInput")
    w1t = nc.dram_tensor("w1t", [FM, P, KD, P], mm_dt, kind="ExternalInput")
    w2 = nc.dram_tensor("w2", [F, D], mm_dt, kind="ExternalInput")
    b1t = nc.dram_tensor("b1t", [P, FM], F32, kind="ExternalInput")
    b2b = nc.dram_tensor("b2b", [P, D], F32, kind="ExternalInput")
    y = nc.dram_tensor("y", [CAP, D], F32, kind="ExternalOutput")

    with tile.TileContext(nc) as tc:
        with ExitStack() as ctx:
            const = ctx.enter_context(tc.tile_pool(name="const", bufs=1))
            xpool = ctx.enter_context(tc.tile_pool(name="x", bufs=1))
            htpool = ctx.enter_context(tc.tile_pool(name="ht", bufs=1))
            w1pool = ctx.enter_context(tc.tile_pool(name="w1", bufs=2))
            w2pool = ctx.enter_context(tc.tile_pool(name="w2", bufs=8))
            ypool = ctx.enter_context(tc.tile_pool(name="yev", bufs=2))
            psum = ctx.enter_context(tc.tile_pool(name="psum", bufs=8, space="PSUM"))

            b1_sb = const.tile([P, FM], F32, name=f"b1_sb_ldw{int(LDW_OPT)}")
            nc.sync.dma_start(b1_sb[:], b1t.ap())
            b2_sb = const.tile([P, D], F32)
            nc.sync.dma_start(b2_sb[:], b2b.ap())

            xt_r = xt.ap().rearrange("(k p) c -> p k c", p=P)  # [128, KD, CAP]
            w1_r = w1t.ap()  # [FM, P, KD, P]
            w2_r = w2.ap()
            y_r = y.ap()

            for t in [t for _ in range(repeat) for t in range(NT)]:
                # w1[fm=0] issued BEFORE the x chunks so the first matmul
                # group isn't queued behind the whole x tile
                w1_next = w1pool.tile([P, KD, P], mm_dt, tag="w1", name="w1p")
                nc.sync.dma_start(w1_next[:], w1_r[0])

                x_sb = xpool.tile([P, KD, TC], mm_dt, tag="x")
                # per-(k, h) chunk DMAs, k-major: matches matmul consumption
                # order and interleaves with weight-stream DMAs so a single
                # monolithic 4MB transfer can't starve the weight queues
                for k in range(KD):
                    for h in range(TC // 512):
                        nc.sync.dma_start(
                            x_sb[:, k, h * 512:(h + 1) * 512],
                            xt_r[:, k, t * TC + h * 512:t * TC + (h + 1) * 512],
                        )

                ht_sb = htpool.tile([P, FM, TC], mm_dt, tag="ht")

                # --- mm1: hT[f_chunk, tok] += W1.T @ xT, fused bias+gelu ---
                # k-outer with both token-half psums live: consecutive matmul
                # pairs share the stationary w1 slice (redundant-LDW elision)
                NH = TC // 512
                for fm in range(FM):
                    w1_sb = w1_next
                    if fm + 1 < FM:
                        w1_next = w1pool.tile([P, KD, P], mm_dt, tag="w1", name="w1p")
                        nc.sync.dma_start(w1_next[:], w1_r[fm + 1])
                    if mm1_pair:
                        ps_h = [
                            psum.tile([P, 512], F32, tag="ps", name="psh")
                            for _ in range(NH)
                        ]
                        for k in range(KD):
                            for h in range(NH):
                                nc.tensor.matmul(
                                    ps_h[h][:],
                                    w1_sb[:, k],
                                    x_sb[:, k, h * 512:(h + 1) * 512],
                                    start=(k == 0),
                                    stop=(k == KD - 1),
                                )
                        for h in range(NH):
                            nc.scalar.activation(
                                ht_sb[:, fm, h * 512:(h + 1) * 512],
                                ps_h[h][:],
                                GELU,
                                bias=b1_sb[:, fm:fm + 1],
                            )
                    else:
                        for h in range(NH):
                            ps = psum.tile([P, 512], F32, tag="ps", name="psh")
                            for k in range(KD):
                                nc.tensor.matmul(
                                    ps[:],
                                    w1_sb[:, k],
                                    x_sb[:, k, h * 512:(h + 1) * 512],
                                    start=(k == 0),
                                    stop=(k == KD - 1),
                                )
                            nc.scalar.activation(
                                ht_sb[:, fm, h * 512:(h + 1) * 512],
                                ps[:],
                                GELU,
                                bias=b1_sb[:, fm:fm + 1],
                            )

                # --- mm2: y[tok, d] += hT.T @ W2, b2 added on eviction ---
                for dh in range(NDH):
                    ps_y = [
                        psum.tile([P, 512], F32, tag="ps", name="psy")
                        for i in range(TM)
                    ]
                    for fm in range(FM):
                        w2_sb = w2pool.tile([P, 512], mm_dt, tag="w2")
                        nc.sync.dma_start(
                            w2_sb[:],
                            w2_r[fm * P:(fm + 1) * P, dh * 512:(dh + 1) * 512],
                        )
                        for tm in range(TM):
                            nc.tensor.matmul(
                                ps_y[tm][:],
                                ht_sb[:, fm, tm * P:(tm + 1) * P],
                                w2_sb[:],
                                start=(fm == 0),
                                stop=(fm == FM - 1),
                            )
                    for tm in range(TM):
                        y_sb = ypool.tile([P, 512], F32, tag="y")
                        nc.vector.tensor_add(
                            y_sb[:], ps_y[tm][:], b2_sb[:, dh * 512:(dh + 1) * 512]
                        )
                        nc.sync.dma_start(
                            y_r[t * TC + tm * P:t * TC + (tm + 1) * P,
                                dh * 512:(dh + 1) * 512],
                            y_sb[:],
                        )

    nc.compile()
    return nc


def _wire_np_dtype(mm_dt):
    if mm_dt == BF16:
        import ml_dtypes

        return ml_dtypes.bfloat16
    return np.float32


def _prep_core_inputs(inputs, W1, b1, W2, b2, e, wdt):
    x_e = inputs[0, e * CAP:(e + 1) * CAP, :]          # [CAP, D]
    xt = np.ascontiguousarray(x_e.T).astype(wdt)       # [D, CAP]
    # W1[e]: [D, F] -> [FM, P(d-part... see kernel), KD, P]
    # kernel reads w1t[fm][p, k, f] == W1[k*P + p, fm*P + f]
    w1t = np.ascontiguousarray(
        W1[e].reshape(KD, P, FM, P).transpose(2, 1, 0, 3)
    ).astype(wdt)
    b1t = np.ascontiguousarray(b1[e].reshape(FM, P).T)  # [P, FM]
    b2b = np.ascontiguousarray(np.broadcast_to(b2[e], (P, D)))
    return {
        "xt": xt,
        "w1t": w1t,
        "w2": np.ascontiguousarray(W2[e]).astype(wdt),
        "b1t": b1t,
        "b2b": b2b,
    }


def get_nc(mm_dt=None, repeat=1, mm1_pair=True):
    if mm_dt is None:
        mm_dt = MM_DT
    key = (mm_dt, repeat, mm1_pair)
    if key not in _cache:
        _cache[key] = _build(mm_dt, repeat, mm1_pair)
    return _cache[key]


def make_in_maps(inputs, W1, b1, W2, b2, mm_dt=None):
    inputs = np.asarray(inputs, dtype=np.float32)
    W1 = np.asarray(W1, dtype=np.float32)
    b1 = np.asarray(b1, dtype=np.float32)
    W2 = np.asarray(W2, dtype=np.float32)
    b2 = np.asarray(b2, dtype=np.float32)
    wdt = _wire_np_dtype(mm_dt if mm_dt is not None else MM_DT)
    return [_prep_core_inputs(inputs, W1, b1, W2, b2, e, wdt) for e in range(E)]


def kernel(inputs, W1, b1, W2, b2):
    nc = get_nc()
    in_maps = make_in_maps(inputs, W1, b1, W2, b2)
    res = run_bass_kernel_spmd(nc, in_maps, list(range(E))).results
    out = np.empty((1, E * CAP, D), dtype=np.float32)
    for e in range(E):
        out[0, e * CAP:(e + 1) * CAP, :] = res[e]["y"]
    return out


if __name__ == "__main__":
    rng = np.random.default_rng(0)
    ins = {
        "inputs": rng.standard_normal((1, E * CAP, D), dtype=np.float32),
        "W1": rng.standard_normal((E, D, F), dtype=np.float32) / np.sqrt(D),
        "b1": np.zeros((E, F), np.float32),
        "W2": rng.standard_normal((E, F, D), dtype=np.float32) / np.sqrt(F),
        "b2": np.zeros((E, D), np.float32),
    }
    y = kernel(**ins)
    print("out", y.shape, y.dtype, float(np.abs(y).mean()))
